# revision 2
# baseline (speedup 1.0000x reference)
"""VQ codebook encoding kernel for Trainium2 (8 NeuronCores, data-parallel over batch).

Per batch b (token count N = H*W, D features, K codes):
  xf = x[b] as (N, D) token-major
  dist[n,k] = s2[k] * (||xf[n]||^2 - 2 xf[n].codes[k] + ||codes[k]||^2)
  a = softmax_k(dist)
  e[b,k,d] = sum_n a[n,k] * xf[n,d] - (sum_n a[n,k]) * codes[k,d]

Design notes (v2):
  - dist is produced token-major directly: for each 128-token tile,
    stationary = x-tile (d-major natural layout), moving = per-d-tile
    codebook columns (128x32).  The s2[k]*||x||^2 term is folded in as
    matmuls of the elementwise-squared x against (s2[k]-s2max), which
    simultaneously shifts the logits so exp never overflows (the shift
    -s2max*||x||^2 is constant across k, so softmax is unchanged).
  - mm2 computes e^T (d-major) with stationary = transposed-x tile and
    moving = a (128x32); colsum(a) via a ones-column stationary.  A small
    PE transpose at the end restores (K, D).
  - x is loaded fp32->bf16 via SWDGE cast DMA, one big DMA per n-chunk.

Sharding: batch B=16 split across 8 cores (2 per core); codes/scale replicated.
"""

import sys

sys.path.insert(0, "/opt/trn_rl_repo")
import numpy as np

import concourse.bass as bass
import concourse.bacc as bacc
import concourse.tile as tile
from concourse import mybir
from concourse.masks import make_identity

FP32 = mybir.dt.float32
BF16 = mybir.dt.bfloat16
FP16 = mybir.dt.float16
AF = mybir.ActivationFunctionType
ALU = mybir.AluOpType
AX = mybir.AxisListType

K = 32
P = 128

B_FULL, D_FULL, H_FULL, W_FULL = 16, 512, 64, 64
N_FULL = H_FULL * W_FULL
NCORES = 8
BS = B_FULL // NCORES

CH = 1024           # tokens per pipeline chunk
TPC = CH // P       # token tiles per chunk (8)
G2 = 2              # token tiles per transpose/copy group


def build(nc, bs=BS, d=D_FULL, n=N_FULL):
    dt_n = d // P                   # 4 d-tiles

    x_d = nc.dram_tensor("x", (bs, d, n), FP32, kind="ExternalInput").ap()
    codes_d = nc.dram_tensor("codes", (K, d), FP32, kind="ExternalInput").ap()
    scale_d = nc.dram_tensor("scale", (K, 1), FP32, kind="ExternalInput").ap()
    e_d = nc.dram_tensor("e", (bs, K, d), FP32, kind="ExternalOutput").ap()

    with tile.TileContext(nc) as tc:
        with (
            tc.tile_pool(name="const", bufs=1) as constp,
            tc.tile_pool(name="xb", bufs=3) as xbp,
            tc.tile_pool(name="sq", bufs=2) as sqp,
            tc.tile_pool(name="xt", bufs=2) as xtp,
            tc.tile_pool(name="smax", bufs=2) as smaxp,
            tc.tile_pool(name="misc", bufs=2) as miscp,
            tc.tile_pool(name="ps_xt", bufs=2, space="PSUM") as ps_xtp,
            tc.tile_pool(name="ps_dist", bufs=2, space="PSUM") as ps_distp,
            tc.tile_pool(name="ps_e", bufs=1, space="PSUM") as ps_ep,
            tc.tile_pool(name="ps_cst", bufs=1, space="PSUM") as ps_cstp,
            tc.tile_pool(name="ps_km", bufs=1, space="PSUM") as ps_kmp,
        ):
            # chunk table: (b, t0, ntok); small first/last chunks shorten
            # pipeline fill and drain.
            per_batch = [512, 1024, 1024, 1024, 512]
            assert sum(per_batch) == n
            chunks = []
            for b in range(bs):
                t0 = 0
                for i, ln in enumerate(per_batch):
                    chunks.append(
                        (b, t0, ln, i == 0, i == len(per_batch) - 1)
                    )
                    t0 += ln
            nch = len(chunks)
            state = {}

            def emit_load(c):
                b, t0, ln, _, _ = chunks[c]
                xb = xbp.tile([P, dt_n, ln], BF16, tag="xb", name=f"xb{c}")
                src = x_d[b, :, t0 : t0 + ln].rearrange("(j p) n -> p j n", p=P)
                nc.gpsimd.dma_start(out=xb, in_=src)
                return xb

            # issue the first loads before any const setup so DMA starts
            # immediately (Pool would otherwise serialize behind identity
            # building).
            loaded = [emit_load(0), emit_load(1)]

            # ---------------- one-time constants ----------------
            codes_sb = constp.tile([K, d], FP32)
            nc.sync.dma_start(out=codes_sb, in_=codes_d)
            scale_col = constp.tile([K, 1], FP32)
            nc.sync.dma_start(out=scale_col, in_=scale_d)

            ident_bf = constp.tile([P, P], BF16)
            make_identity(nc, ident_bf)
            ident_f32 = constp.tile([P, P], FP32)
            make_identity(nc, ident_f32)

            ones_row = constp.tile([1, P], BF16)
            nc.vector.memset(ones_row, 1.0)
            ones_col = constp.tile([P, 1], BF16)
            nc.vector.memset(ones_col, 1.0)

            s2_col = constp.tile([K, 1], FP32)
            nc.vector.tensor_mul(s2_col, scale_col, scale_col)
            sq_codes = constp.tile([K, d], FP32)
            c2_col = constp.tile([K, 1], FP32)
            nc.scalar.activation(
                out=sq_codes, in_=codes_sb, func=AF.Square, accum_out=c2_col
            )
            s2c2_col = constp.tile([K, 1], FP32)
            nc.vector.tensor_mul(s2c2_col, s2_col, c2_col)
            neg2s2_col = constp.tile([K, 1], FP32)
            nc.vector.tensor_scalar_mul(neg2s2_col, s2_col, -2.0)

            # w_kn[k, d] = -2 s2[k] codes[k, d]
            w_kn = constp.tile([K, d], BF16)
            nc.vector.tensor_scalar_mul(w_kn, codes_sb, neg2s2_col)

            # transpose rows -> single-partition rows: s2_row, s2c2_row
            ps_small = ps_cstp.tile([1, 2 * K], FP32, tag="cst", name="ps_small")
            nc.tensor.transpose(ps_small[:, :K], s2_col, ident_f32[:K, :K])
            nc.tensor.transpose(ps_small[:, K : 2 * K], s2c2_col, ident_f32[:K, :K])
            s2_row = constp.tile([1, K], FP32)
            nc.vector.tensor_copy(s2_row, ps_small[:, :K])
            s2c2_row = constp.tile([1, K], BF16)
            nc.vector.tensor_copy(s2c2_row, ps_small[:, K : 2 * K])

            s2max = constp.tile([1, 1], FP32)
            nc.vector.tensor_reduce(s2max, s2_row, axis=AX.X, op=ALU.max)
            s2m_row = constp.tile([1, K], FP16)
            nc.vector.tensor_scalar(
                s2m_row, in0=s2_row, scalar1=s2max, scalar2=None, op0=ALU.subtract
            )

            # w_dK[:, j, :] = w_kn[:, jP:(j+1)P].T   (d-major codebook cols)
            # codesT[:, j, :] = codes[:, jP:(j+1)P].T  (fp32, for final combine)
            ps_w = ps_cstp.tile([P, dt_n, K], BF16, tag="cst", name="ps_w")
            for j in range(dt_n):
                nc.tensor.transpose(
                    ps_w[:, j, :], w_kn[:, j * P : (j + 1) * P], ident_bf[:K, :K]
                )
            w_dK = constp.tile([P, dt_n, K], BF16)
            nc.vector.tensor_copy(w_dK, ps_w)

            # s2mb[dpart, k] = s2[k] - s2max, broadcast down partitions via PE
            ps_s2mb = ps_cstp.tile([P, K], FP32, tag="cst", name="ps_s2mb")
            nc.tensor.matmul(ps_s2mb, ones_row, s2m_row, start=True, stop=True)
            s2mb = constp.tile([P, K], FP16)
            nc.vector.tensor_copy(s2mb, ps_s2mb)

            neg_ones_col = constp.tile([P, 1], BF16)
            nc.vector.memset(neg_ones_col, -1.0)
            zeros_row = constp.tile([1, P], BF16)
            nc.vector.memset(zeros_row, 0.0)

            # ---------------- pipelined chunk loop ----------------

            def emit_square(c, xb):
                b, t0, ln, _, _ = chunks[c]
                sq = sqp.tile([P, dt_n, ln], FP16, tag="sq", name=f"sq{c}")
                # split j-blocks across DVE/DVE/ACT/Pool for balance
                nc.vector.tensor_mul(sq[:, :2, :], xb[:, :2, :], xb[:, :2, :])
                nc.scalar.activation(sq[:, 2, :], xb[:, 2, :], AF.Square)
                nc.gpsimd.tensor_mul(sq[:, 3, :], xb[:, 3, :], xb[:, 3, :])
                return sq

            COPY_ENG = ["scalar", "vector"]
            copy_ctr = [0]

            def emit_transposes(c, xb):
                b, t0, ln, _, _ = chunks[c]
                tpc = ln // P
                xt = xtp.tile([P, tpc, d], BF16, tag="xt", name=f"xt{c}")
                for g in range(tpc // G2):
                    psx = ps_xtp.tile([P, G2, d], BF16, tag="psx", name=f"psx{c}_{g}")
                    for tt in range(G2):
                        t = g * G2 + tt
                        for j in range(dt_n):
                            nc.tensor.transpose(
                                psx[:, tt, j * P : (j + 1) * P],
                                xb[:, j, t * P : (t + 1) * P],
                                ident_bf,
                            )
                    eng_name = COPY_ENG[copy_ctr[0] % len(COPY_ENG)]
                    copy_ctr[0] += 1
                    if eng_name == "scalar":
                        nc.scalar.copy(xt[:, g * G2 : (g + 1) * G2, :], psx)
                    else:
                        nc.vector.tensor_copy(xt[:, g * G2 : (g + 1) * G2, :], psx)
                return xt

            def emit_mm1(c, xb, sq):
                b, t0, ln, _, _ = chunks[c]
                tpc = ln // P
                psd = ps_distp.tile([P, tpc, K], FP32, tag="dist", name=f"psd{c}")
                for t in range(tpc):
                    sl = slice(t * P, (t + 1) * P)
                    for j in range(dt_n):
                        nc.tensor.matmul(
                            psd[:, t, :],
                            xb[:, j, sl],
                            w_dK[:, j, :],
                            start=(j == 0),
                            stop=False,
                        )
                    for j in range(dt_n):
                        nc.tensor.matmul(
                            psd[:, t, :], sq[:, j, sl], s2mb, start=False, stop=False
                        )
                    nc.tensor.matmul(
                        psd[:, t, :], ones_row, s2c2_row, start=False, stop=True
                    )
                return psd

            def emit_softmax(c, psd):
                b, t0, ln, _, _ = chunks[c]
                tpc = ln // P
                pexp = smaxp.tile([P, tpc, K], FP16, tag="pexp", name=f"pexp{c}")
                nc.scalar.activation(pexp, psd, AF.Exp)
                scol = smaxp.tile([P, tpc, 1], FP32, tag="scol", name=f"scol{c}")
                nc.vector.tensor_reduce(scol, pexp, axis=AX.X, op=ALU.add)
                rcol = smaxp.tile([P, tpc, 1], FP32, tag="rcol", name=f"rcol{c}")
                nc.vector.reciprocal(rcol, scol)
                a_sb = smaxp.tile([P, tpc, K], FP16, tag="a", name=f"a{c}")
                nc.vector.tensor_mul(a_sb, pexp, rcol[:].to_broadcast([P, tpc, K]))
                return a_sb

            def emit_mm2(c, xt, a_sb):
                b, t0, ln, firstc, lastc = chunks[c]
                tpc = ln // P
                if firstc:
                    state["pse"] = ps_ep.tile(
                        [P, dt_n, K], FP32, tag="e1T", name="pse"
                    )
                    state["psc"] = ps_ep.tile([K, 1], FP32, tag="cs", name="psc")
                    # pse[:, j, :] are 4 interleaved accumulation groups in one
                    # bank; a per-group start=True would clear the whole bank's
                    # has_written bits.  Zero-fill once, then always accumulate.
                    nc.tensor.matmul(
                        state["pse"][:],
                        zeros_row,
                        ones_row[:, : dt_n * K],
                        start=True,
                        stop=False,
                        skip_group_check=True,
                    )
                pse, psc = state["pse"], state["psc"]
                for t in range(tpc):
                    first = firstc and t == 0
                    stop = lastc and t == tpc - 1
                    for j in range(dt_n):
                        nc.tensor.matmul(
                            pse[:, j, :],
                            xt[:, t, j * P : (j + 1) * P],
                            a_sb[:, t, :],
                            start=False,
                            stop=stop and j == dt_n - 1,
                            skip_group_check=True,
                        )
                    nc.tensor.matmul(
                        psc, a_sb[:, t, :], neg_ones_col, start=first, stop=stop
                    )

            def emit_combine(b):
                pse, psc = state["pse"], state["psc"]
                ncs_col = miscp.tile([K, 1], FP32, tag="ncs")
                nc.vector.tensor_copy(ncs_col, psc)
                eT_sb = miscp.tile([P, dt_n, K], FP32, tag="eT")
                nc.vector.tensor_copy(eT_sb, pse)
                ps_km = ps_kmp.tile([K, d], FP32, tag="ekm")
                for j in range(dt_n):
                    nc.tensor.transpose(
                        ps_km[:, j * P : (j + 1) * P], eT_sb[:, j, :], ident_f32
                    )
                # e = ps_km + (-cs) * codes
                e_sb = miscp.tile([K, d], FP32, tag="esb")
                nc.vector.scalar_tensor_tensor(
                    out=e_sb,
                    in0=codes_sb,
                    scalar=ncs_col,
                    in1=ps_km,
                    op0=ALU.mult,
                    op1=ALU.add,
                )
                nc.sync.dma_start(out=e_d[b], in_=e_sb)

            def emit_tail(c, xt, psd):
                # last chunk: softmax and mm2 interleaved per 2-tile group to
                # shorten the pipeline drain.
                b, t0, ln, firstc, lastc = chunks[c]
                tpc = ln // P
                pse, psc = state["pse"], state["psc"]
                gsz = 2
                for g0 in range(0, tpc, gsz):
                    gn = min(gsz, tpc - g0)
                    sl = slice(g0, g0 + gn)
                    pexp = smaxp.tile([P, gn, K], FP16, tag="pexp", name=f"tp{g0}")
                    nc.scalar.activation(pexp, psd[:, sl, :], AF.Exp)
                    scol = smaxp.tile([P, gn, 1], FP32, tag="scol", name=f"ts{g0}")
                    nc.vector.tensor_reduce(scol, pexp, axis=AX.X, op=ALU.add)
                    rcol = smaxp.tile([P, gn, 1], FP32, tag="rcol", name=f"tr{g0}")
                    nc.vector.reciprocal(rcol, scol)
                    a_sb = smaxp.tile([P, gn, K], FP16, tag="a", name=f"ta{g0}")
                    nc.vector.tensor_mul(
                        a_sb, pexp, rcol[:].to_broadcast([P, gn, K])
                    )
                    for tt in range(gn):
                        t = g0 + tt
                        stop = lastc and t == tpc - 1
                        for j in range(dt_n):
                            nc.tensor.matmul(
                                pse[:, j, :],
                                xt[:, t, j * P : (j + 1) * P],
                                a_sb[:, tt, :],
                                start=False,
                                stop=stop and j == dt_n - 1,
                                skip_group_check=True,
                            )
                        nc.tensor.matmul(
                            psc, a_sb[:, tt, :], neg_ones_col, start=False, stop=stop
                        )

            # pipeline: loads run ahead via pool bufs; PE order per chunk is
            # transposes(c), mm1(c), mm2(c-1) so PE never waits on softmax.
            prev = None  # (c, xt, a_sb)
            for c in range(nch):
                xb = loaded[c]
                if c + 2 < nch:
                    loaded.append(emit_load(c + 2))
                sq = emit_square(c, xb)
                xt = emit_transposes(c, xb)
                psd = emit_mm1(c, xb, sq)
                if prev is not None:
                    emit_mm2(prev[0], prev[1], prev[2])
                    if chunks[prev[0]][4]:
                        emit_combine(chunks[prev[0]][0])
                a_sb = emit_softmax(c, psd)
                prev = (c, xt, a_sb)
            emit_mm2(prev[0], prev[1], prev[2])
            emit_combine(chunks[nch - 1][0])

_CACHE = {}


def _get_compiled():
    if "nc" not in _CACHE:
        nc = bacc.Bacc("TRN2", target_bir_lowering=False, debug=False)
        build(nc)
        nc.compile()
        _CACHE["nc"] = nc
    return _CACHE["nc"]


def kernel(x, codes, scale):
    from concourse import bass_utils

    b_total = x.shape[0]
    bs = b_total // NCORES
    xr = np.ascontiguousarray(x.reshape(b_total, x.shape[1], -1), dtype=np.float32)
    codes_c = np.ascontiguousarray(codes, dtype=np.float32)
    scale_c = np.ascontiguousarray(scale, dtype=np.float32).reshape(K, 1)

    nc = _get_compiled()
    in_maps = [
        {"x": xr[i * bs : (i + 1) * bs], "codes": codes_c, "scale": scale_c}
        for i in range(NCORES)
    ]
    res = bass_utils.run_bass_kernel_spmd(nc, in_maps, core_ids=list(range(NCORES)))
    e = np.concatenate([r["e"] for r in res.results], axis=0)
    return e.astype(np.float32)


# revision 3
# speedup vs baseline: 1.0245x; 1.0245x over previous
"""VQ codebook encoding kernel for Trainium2 (8 NeuronCores, data-parallel over batch).

Per batch b (token count N = H*W, D features, K codes):
  xf = x[b] as (N, D) token-major
  dist[n,k] = s2[k] * (||xf[n]||^2 - 2 xf[n].codes[k] + ||codes[k]||^2)
  a = softmax_k(dist)
  e[b,k,d] = sum_n a[n,k] * xf[n,d] - (sum_n a[n,k]) * codes[k,d]

Design notes (v2):
  - dist is produced token-major directly: for each 128-token tile,
    stationary = x-tile (d-major natural layout), moving = per-d-tile
    codebook columns (128x32).  The s2[k]*||x||^2 term is folded in as
    matmuls of the elementwise-squared x against (s2[k]-s2max), which
    simultaneously shifts the logits so exp never overflows (the shift
    -s2max*||x||^2 is constant across k, so softmax is unchanged).
  - mm2 computes e^T (d-major) with stationary = transposed-x tile and
    moving = a (128x32); colsum(a) via a ones-column stationary.  A small
    PE transpose at the end restores (K, D).
  - x is loaded fp32->bf16 via SWDGE cast DMA, one big DMA per n-chunk.

Sharding: batch B=16 split across 8 cores (2 per core); codes/scale replicated.
"""

import sys

sys.path.insert(0, "/opt/trn_rl_repo")
import numpy as np

import concourse.bass as bass
import concourse.bacc as bacc
import concourse.tile as tile
from concourse import mybir
from concourse.masks import make_identity

FP32 = mybir.dt.float32
BF16 = mybir.dt.bfloat16
FP16 = mybir.dt.float16
AF = mybir.ActivationFunctionType
ALU = mybir.AluOpType
AX = mybir.AxisListType

K = 32
P = 128

B_FULL, D_FULL, H_FULL, W_FULL = 16, 512, 64, 64
N_FULL = H_FULL * W_FULL
NCORES = 8
BS = B_FULL // NCORES

CH = 1024           # tokens per pipeline chunk
TPC = CH // P       # token tiles per chunk (8)
G2 = 2              # token tiles per transpose/copy group


def build(nc, bs=BS, d=D_FULL, n=N_FULL):
    dt_n = d // P                   # 4 d-tiles

    x_d = nc.dram_tensor("x", (bs, d, n), FP32, kind="ExternalInput").ap()
    codes_d = nc.dram_tensor("codes", (K, d), FP32, kind="ExternalInput").ap()
    scale_d = nc.dram_tensor("scale", (K, 1), FP32, kind="ExternalInput").ap()
    e_d = nc.dram_tensor("e", (bs, K, d), FP32, kind="ExternalOutput").ap()

    with tile.TileContext(nc) as tc:
        with (
            tc.tile_pool(name="const", bufs=1) as constp,
            tc.tile_pool(name="xb", bufs=3) as xbp,
            tc.tile_pool(name="sq", bufs=2) as sqp,
            tc.tile_pool(name="xt", bufs=3) as xtp,
            tc.tile_pool(name="smax", bufs=3) as smaxp,
            tc.tile_pool(name="misc", bufs=2) as miscp,
            tc.tile_pool(name="ps_xt", bufs=2, space="PSUM") as ps_xtp,
            tc.tile_pool(name="ps_dist", bufs=2, space="PSUM") as ps_distp,
            tc.tile_pool(name="ps_e", bufs=2, space="PSUM") as ps_ep,
            tc.tile_pool(name="ps_cst", bufs=1, space="PSUM") as ps_cstp,
            tc.tile_pool(name="ps_km", bufs=1, space="PSUM") as ps_kmp,
        ):
            # chunk table: (b, t0, ntok); small first/last chunks shorten
            # pipeline fill and drain.
            per_batch = [512, 1024, 1024, 1024, 512]
            assert sum(per_batch) == n
            chunks = []
            for b in range(bs):
                t0 = 0
                for i, ln in enumerate(per_batch):
                    chunks.append(
                        (b, t0, ln, i == 0, i == len(per_batch) - 1)
                    )
                    t0 += ln
            nch = len(chunks)
            state = {}

            def emit_load(c):
                b, t0, ln, _, _ = chunks[c]
                xb = xbp.tile([P, dt_n, ln], BF16, tag="xb", name=f"xb{c}")
                src = x_d[b, :, t0 : t0 + ln].rearrange("(j p) n -> p j n", p=P)
                nc.gpsimd.dma_start(out=xb, in_=src)
                return xb

            # issue the first loads before any const setup so DMA starts
            # immediately (Pool would otherwise serialize behind identity
            # building).
            loaded = [emit_load(0), emit_load(1)]

            # ---------------- one-time constants ----------------
            codes_sb = constp.tile([K, d], FP32)
            nc.sync.dma_start(out=codes_sb, in_=codes_d)
            scale_col = constp.tile([K, 1], FP32)
            nc.sync.dma_start(out=scale_col, in_=scale_d)

            ident_bf = constp.tile([P, P], BF16)
            make_identity(nc, ident_bf)
            ident_f32 = constp.tile([P, P], FP32)
            make_identity(nc, ident_f32)

            ones_row = constp.tile([1, P + 8], BF16)
            nc.vector.memset(ones_row, 1.0)
            ones_col = constp.tile([P, 1], BF16)
            nc.vector.memset(ones_col, 1.0)

            s2_col = constp.tile([K, 1], FP32)
            nc.vector.tensor_mul(s2_col, scale_col, scale_col)
            sq_codes = constp.tile([K, d], FP32)
            c2_col = constp.tile([K, 1], FP32)
            nc.scalar.activation(
                out=sq_codes, in_=codes_sb, func=AF.Square, accum_out=c2_col
            )
            s2c2_col = constp.tile([K, 1], FP32)
            nc.vector.tensor_mul(s2c2_col, s2_col, c2_col)
            neg2s2_col = constp.tile([K, 1], FP32)
            nc.vector.tensor_scalar_mul(neg2s2_col, s2_col, -2.0)

            # w_kn[k, d] = -2 s2[k] codes[k, d]
            w_kn = constp.tile([K, d], BF16)
            nc.vector.tensor_scalar_mul(w_kn, codes_sb, neg2s2_col)

            # transpose rows -> single-partition rows: s2_row, s2c2_row
            ps_small = ps_cstp.tile([1, 2 * K], FP32, tag="cst", name="ps_small")
            nc.tensor.transpose(ps_small[:, :K], s2_col, ident_f32[:K, :K])
            nc.tensor.transpose(ps_small[:, K : 2 * K], s2c2_col, ident_f32[:K, :K])
            s2_row = constp.tile([1, K], FP32)
            nc.vector.tensor_copy(s2_row, ps_small[:, :K])
            s2c2_row = constp.tile([1, K], BF16)
            nc.vector.tensor_copy(s2c2_row, ps_small[:, K : 2 * K])

            s2max = constp.tile([1, 1], FP32)
            nc.vector.tensor_reduce(s2max, s2_row, axis=AX.X, op=ALU.max)
            s2m_row = constp.tile([1, K], FP16)
            nc.vector.tensor_scalar(
                s2m_row, in0=s2_row, scalar1=s2max, scalar2=None, op0=ALU.subtract
            )

            # w_dK[:, j, :] = w_kn[:, jP:(j+1)P].T   (d-major codebook cols)
            # codesT[:, j, :] = codes[:, jP:(j+1)P].T  (fp32, for final combine)
            ps_w = ps_cstp.tile([P, dt_n, K], BF16, tag="cst", name="ps_w")
            for j in range(dt_n):
                nc.tensor.transpose(
                    ps_w[:, j, :], w_kn[:, j * P : (j + 1) * P], ident_bf[:K, :K]
                )
            w_dK = constp.tile([P, dt_n, K], BF16)
            nc.vector.tensor_copy(w_dK, ps_w)

            # s2mb[dpart, k] = s2[k] - s2max, broadcast down partitions via PE
            ps_s2mb = ps_cstp.tile([P, K], FP32, tag="cst", name="ps_s2mb")
            nc.tensor.matmul(ps_s2mb, ones_row[:, :P], s2m_row, start=True, stop=True)
            s2mb = constp.tile([P, K], FP16)
            nc.vector.tensor_copy(s2mb, ps_s2mb)

            neg_ones_col = constp.tile([P, 1], BF16)
            nc.vector.memset(neg_ones_col, -1.0)
            zeros_row = constp.tile([1, P], BF16)
            nc.vector.memset(zeros_row, 0.0)

            # ---------------- pipelined chunk loop ----------------

            def emit_square(c, xb):
                b, t0, ln, _, _ = chunks[c]
                sq = sqp.tile([P, dt_n, ln], FP16, tag="sq", name=f"sq{c}")
                # split j-blocks across DVE/DVE/ACT/Pool for balance
                nc.vector.tensor_mul(sq[:, :2, :], xb[:, :2, :], xb[:, :2, :])
                nc.scalar.activation(sq[:, 2, :], xb[:, 2, :], AF.Square)
                nc.gpsimd.tensor_mul(sq[:, 3, :], xb[:, 3, :], xb[:, 3, :])
                return sq

            COPY_ENG = ["scalar", "vector"]
            copy_ctr = [0]

            def emit_transposes(c, xb):
                b, t0, ln, _, _ = chunks[c]
                tpc = ln // P
                xt = xtp.tile([P, tpc, d], BF16, tag="xt", name=f"xt{c}")
                for g in range(tpc // G2):
                    psx = ps_xtp.tile([P, G2, d], BF16, tag="psx", name=f"psx{c}_{g}")
                    for tt in range(G2):
                        t = g * G2 + tt
                        for j in range(dt_n):
                            nc.tensor.transpose(
                                psx[:, tt, j * P : (j + 1) * P],
                                xb[:, j, t * P : (t + 1) * P],
                                ident_bf,
                            )
                    eng_name = COPY_ENG[copy_ctr[0] % len(COPY_ENG)]
                    copy_ctr[0] += 1
                    if eng_name == "scalar":
                        nc.scalar.copy(xt[:, g * G2 : (g + 1) * G2, :], psx)
                    else:
                        nc.vector.tensor_copy(xt[:, g * G2 : (g + 1) * G2, :], psx)
                return xt

            def emit_mm1(c, xb, sq):
                b, t0, ln, _, _ = chunks[c]
                tpc = ln // P
                psd = ps_distp.tile([P, tpc, K], FP32, tag="dist", name=f"psd{c}")
                for t in range(tpc):
                    sl = slice(t * P, (t + 1) * P)
                    for j in range(dt_n):
                        nc.tensor.matmul(
                            psd[:, t, :],
                            xb[:, j, sl],
                            w_dK[:, j, :],
                            start=(j == 0),
                            stop=False,
                        )
                    for j in range(dt_n):
                        nc.tensor.matmul(
                            psd[:, t, :], sq[:, j, sl], s2mb, start=False, stop=False
                        )
                    nc.tensor.matmul(
                        psd[:, t, :], ones_row[:, :P], s2c2_row, start=False, stop=True
                    )
                return psd

            def emit_softmax(c, psd):
                b, t0, ln, _, _ = chunks[c]
                tpc = ln // P
                pexp = smaxp.tile([P, tpc, K], FP16, tag="pexp", name=f"pexp{c}")
                nc.scalar.activation(pexp, psd, AF.Exp)
                scol = smaxp.tile([P, tpc, 1], FP32, tag="scol", name=f"scol{c}")
                nc.vector.tensor_reduce(scol, pexp, axis=AX.X, op=ALU.add)
                rcol = smaxp.tile([P, tpc, 1], FP32, tag="rcol", name=f"rcol{c}")
                nc.vector.reciprocal(rcol, scol)
                a_sb = smaxp.tile([P, tpc, K], FP16, tag="a", name=f"a{c}")
                nc.vector.tensor_mul(a_sb, pexp, rcol[:].to_broadcast([P, tpc, K]))
                return a_sb

            def emit_mm2(c, xt, a_sb):
                b, t0, ln, firstc, lastc = chunks[c]
                tpc = ln // P
                if firstc:
                    # pse[:, jK:(j+1)K] are 4 interleaved accumulation groups
                    # in one bank; a per-group start=True would clear the whole
                    # bank's has_written bits.  Zero-fill once, then always
                    # accumulate (start=False everywhere).
                    pext = ps_ep.tile(
                        [P, dt_n * K], FP32, tag="e1T", name=f"pse{b}"
                    )
                    state[b] = pext
                    state[(b, "cs")] = ps_cstp.tile(
                        [K, 1], FP32, tag="cst", name=f"psc{b}"
                    )
                    nc.tensor.matmul(
                        pext,
                        zeros_row,
                        ones_row[:, : dt_n * K],
                        start=True,
                        stop=False,
                        skip_group_check=True,
                    )
                pext = state[b]
                psc = state[(b, "cs")]
                for t in range(tpc):
                    stop = lastc and t == tpc - 1
                    for j in range(dt_n):
                        nc.tensor.matmul(
                            pext[:, j * K : (j + 1) * K],
                            xt[:, t, j * P : (j + 1) * P],
                            a_sb[:, t, :],
                            start=False,
                            stop=stop and j == dt_n - 1,
                            skip_group_check=True,
                        )
                    nc.tensor.matmul(
                        psc,
                        a_sb[:, t, :],
                        neg_ones_col,
                        start=(firstc and t == 0),
                        stop=stop,
                    )

            def emit_combine(b):
                pext = state[b]
                psc = state[(b, "cs")]
                ncs_col = miscp.tile([K, 1], FP32, tag="ncs")
                nc.vector.tensor_copy(ncs_col, psc)
                eT_sb = miscp.tile([P, dt_n, K], FP32, tag="eT")
                nc.vector.tensor_copy(eT_sb, pext[:, : dt_n * K])
                ps_km = ps_kmp.tile([K, d], FP32, tag="ekm")
                for j in range(dt_n):
                    nc.tensor.transpose(
                        ps_km[:, j * P : (j + 1) * P], eT_sb[:, j, :], ident_f32
                    )
                # e = ps_km + (-cs) * codes
                e_sb = miscp.tile([K, d], FP32, tag="esb")
                nc.vector.scalar_tensor_tensor(
                    out=e_sb,
                    in0=codes_sb,
                    scalar=ncs_col,
                    in1=ps_km,
                    op0=ALU.mult,
                    op1=ALU.add,
                )
                nc.sync.dma_start(out=e_d[b], in_=e_sb)

            def emit_tail(c, xt, psd):
                # last chunk: softmax and mm2 interleaved per 2-tile group to
                # shorten the pipeline drain.
                b, t0, ln, firstc, lastc = chunks[c]
                tpc = ln // P
                pse, psc = state["pse"], state["psc"]
                gsz = 2
                for g0 in range(0, tpc, gsz):
                    gn = min(gsz, tpc - g0)
                    sl = slice(g0, g0 + gn)
                    pexp = smaxp.tile([P, gn, K], FP16, tag="pexp", name=f"tp{g0}")
                    nc.scalar.activation(pexp, psd[:, sl, :], AF.Exp)
                    scol = smaxp.tile([P, gn, 1], FP32, tag="scol", name=f"ts{g0}")
                    nc.vector.tensor_reduce(scol, pexp, axis=AX.X, op=ALU.add)
                    rcol = smaxp.tile([P, gn, 1], FP32, tag="rcol", name=f"tr{g0}")
                    nc.vector.reciprocal(rcol, scol)
                    a_sb = smaxp.tile([P, gn, K], FP16, tag="a", name=f"ta{g0}")
                    nc.vector.tensor_mul(
                        a_sb, pexp, rcol[:].to_broadcast([P, gn, K])
                    )
                    for tt in range(gn):
                        t = g0 + tt
                        stop = lastc and t == tpc - 1
                        for j in range(dt_n):
                            nc.tensor.matmul(
                                pse[:, j, :],
                                xt[:, t, j * P : (j + 1) * P],
                                a_sb[:, tt, :],
                                start=False,
                                stop=stop and j == dt_n - 1,
                                skip_group_check=True,
                            )
                        nc.tensor.matmul(
                            psc, a_sb[:, tt, :], neg_ones_col, start=False, stop=stop
                        )

            # pipeline: loads run ahead via pool bufs; PE order per chunk is
            # transposes(c), mm1(c), mm2(c-1) so PE never waits on softmax.
            pending = []  # (c, xt, a_sb)
            for c in range(nch):
                xb = loaded[c]
                if c + 2 < nch:
                    loaded.append(emit_load(c + 2))
                sq = emit_square(c, xb)
                xt = emit_transposes(c, xb)
                psd = emit_mm1(c, xb, sq)
                if len(pending) >= 2:
                    p = pending.pop(0)
                    emit_mm2(p[0], p[1], p[2])
                    if chunks[p[0]][4]:
                        emit_combine(chunks[p[0]][0])
                a_sb = emit_softmax(c, psd)
                pending.append((c, xt, a_sb))
            for p in pending:
                emit_mm2(p[0], p[1], p[2])
                if chunks[p[0]][4]:
                    emit_combine(chunks[p[0]][0])

_CACHE = {}


def _get_compiled():
    if "nc" not in _CACHE:
        nc = bacc.Bacc("TRN2", target_bir_lowering=False, debug=False)
        build(nc)
        nc.compile()
        _CACHE["nc"] = nc
    return _CACHE["nc"]


def kernel(x, codes, scale):
    from concourse import bass_utils

    b_total = x.shape[0]
    bs = b_total // NCORES
    xr = np.ascontiguousarray(x.reshape(b_total, x.shape[1], -1), dtype=np.float32)
    codes_c = np.ascontiguousarray(codes, dtype=np.float32)
    scale_c = np.ascontiguousarray(scale, dtype=np.float32).reshape(K, 1)

    nc = _get_compiled()
    in_maps = [
        {"x": xr[i * bs : (i + 1) * bs], "codes": codes_c, "scale": scale_c}
        for i in range(NCORES)
    ]
    res = bass_utils.run_bass_kernel_spmd(nc, in_maps, core_ids=list(range(NCORES)))
    e = np.concatenate([r["e"] for r in res.results], axis=0)
    return e.astype(np.float32)
